# revision 1
# baseline (speedup 1.0000x reference)
"""Trainium2 Bass kernel for policy-weighted multi-head attention.

Reference computation (per batch b, 8 batches):
    qkv = x @ qkv_w.T                     # [N, 3*H*HD]
    q, k, v per head                      # H=12 heads, HD=64
    s = (q * HD^-0.5) @ k.T               # [N, N]
    a[n,m] ~ exp(s[n,m]) * (pol[m] + (1-pol[m])*eye)  normalized over m
    out = a @ v ; y = out @ proj_w.T + b

Sharding: pure data parallel, one batch per NeuronCore (8 cores).

Kernel strategy (per core):
  - Host pre-transposes x, qkv_w, proj_w so no on-chip transposes are needed.
  - All matmuls run as float32r (full fp32 data, streams ~250ns per
    [128x128x512] on the PE; ~2e-5 mean rel err per 128-dot).
  - Attention runs in the S^T layout (partitions = key index m): the softmax
    sum over m folds into the PE via an appended ones column on the
    (policy-prescaled) V; the denominator appears as row 64 of the
    attention-output matmul.
  - The policy multiply is folded into V (rows pre-scaled by pol[m]); the
    diagonal term becomes masked multiplies with a precomputed
    [128, 8, 128] mask whose diagonal is 1/pol.
  - Head-major processing with rotating qk buffers: the qk^T matmuls for
    head pair j+1 are emitted (and their weights DMA'd) while pair j's
    attention runs, so only 3 q/k pair buffers are resident.
  - The per-head dataflow is software-pipelined for the in-order engine
    queues: S^T matmuls for chunk-pair t overlap the exp of pair t-1 and
    the a@v matmuls of pair t-2.
  - Denominator reciprocals are batched (3 heads x 2 halves at a time:
    gathered onto 6 partitions by SBUF->SBUF DMA, one DVE reciprocal,
    broadcast back via a DRAM bounce -- DMA cannot partition-broadcast
    from SBUF).
  - max-subtraction and the eps terms of the reference softmax are dropped:
    logits are ~N(0,1) so exp() cannot overflow, and the eps corrections
    are ~1e-9 relative -- far below fp32 noise.
"""

import os

os.environ.setdefault("JAX_PLATFORMS", "axon")

from contextlib import ExitStack

import ml_dtypes
import numpy as np

import concourse.bass as bass
import concourse.tile as tile
from concourse import bacc, mybir
from concourse.bass_utils import run_bass_kernel_spmd

B, N, C = 8, 1024, 768
H, HD = 12, 64
SCALE = HD ** (-0.5)
F32 = mybir.dt.float32
F32R = mybir.dt.float32r
BF16 = mybir.dt.bfloat16
P = 128
NC_ = N // P  # 8 seq chunks
CC = C // P  # 6 channel chunks
NH = N // 512  # 2 free-dim halves of the seq axis

LAST_RESULTS = None  # BassKernelResults of the most recent run (for test.py)


def _build_nc():
    nc = bacc.Bacc(None, target_bir_lowering=False)

    xT_d = nc.dram_tensor("xT", [C, N], F32R, kind="ExternalInput")
    wqkT_d = nc.dram_tensor("wqkT", [C, 2 * H * HD], F32R, kind="ExternalInput")
    wvT_d = nc.dram_tensor("wvT", [C, H * HD], F32R, kind="ExternalInput")
    pwT_d = nc.dram_tensor("pwT", [C, C], F32R, kind="ExternalInput")
    bias_d = nc.dram_tensor("bias", [C], F32, kind="ExternalInput")
    polT_d = nc.dram_tensor("polT", [P, NC_], F32, kind="ExternalInput")
    dmask_d = nc.dram_tensor("dmask", [P, NC_, P], F32R, kind="ExternalInput")
    y_d = nc.dram_tensor("y", [N, C], F32, kind="ExternalOutput")

    with ExitStack() as ctx:
        tc = ctx.enter_context(tile.TileContext(nc))

        persist = ctx.enter_context(tc.tile_pool(name="persist", bufs=1))
        xT_sb = persist.tile([P, CC, N], F32R)
        # v in natural layout, pol-scaled, with a pol column at d=64
        v_aug = persist.tile([P, NC_, H, HD + 1], F32R)
        pw_sb = persist.tile([P, CC, C], F32R)
        b_sb = persist.tile([P, C], F32)
        polT_sb = persist.tile([P, NC_], F32)
        dmask_sb = persist.tile([P, NC_, P], F32R)

        nc.sync.dma_start(out=polT_sb, in_=polT_d[:])
        nc.sync.dma_start(out=dmask_sb, in_=dmask_d[:])
        nc.sync.dma_start(out=b_sb, in_=bias_d[:].partition_broadcast(P))
        for cc in range(CC):
            nc.sync.dma_start(
                out=xT_sb[:, cc, 0:512], in_=xT_d[cc * P : (cc + 1) * P, 0:512]
            )
        # rotating q/k buffers: one pair j holds q chunk j and k chunk j+6
        qkp = ctx.enter_context(tc.tile_pool(name="qkp", bufs=1))
        ps_qk = ctx.enter_context(tc.tile_pool(name="ps_qk", bufs=1, space="PSUM"))
        ps_st = ctx.enter_context(tc.tile_pool(name="ps_st", bufs=3, space="PSUM"))
        ps_av = ctx.enter_context(tc.tile_pool(name="ps_av", bufs=1, space="PSUM"))
        rdramp = ctx.enter_context(tc.tile_pool(name="rdram", bufs=2, space="DRAM"))

        def emit_qk_pair(jq, nhs=(0, 1), tiles=None):
            """Load wqk columns for pair jq and run its qk^T matmul groups
            for the given seq halves.

            Returns (qk_t, wqk_t); qk_t is the [P, 2, N] tile: [:,0,:] =
            q-chunk jq, [:,1,:] = k-chunk jq+6 (e on partitions)."""
            if tiles is None:
                wqk_t = qkp.tile(
                    [P, CC, 2, P], F32R, tag="wqk", bufs=3, name="wqk_t"
                )
                for kk, j in ((0, jq), (1, jq + CC)):
                    for cc in range(CC):
                        nc.sync.dma_start(
                            out=wqk_t[:, cc, kk, :],
                            in_=wqkT_d[cc * P : (cc + 1) * P, j * P : (j + 1) * P],
                        )
                qk_t = qkp.tile([P, 2, N], F32R, tag="qkT", bufs=3, name="qk_t")
            else:
                qk_t, wqk_t = tiles
            with nc.named_scope("qk_mm"):
                for kk in range(2):
                    for nh in nhs:
                        ps = ps_qk.tile([P, 512], F32, tag="ps_qk", name="ps_qk")
                        for cc in range(CC):
                            nc.tensor.matmul(
                                ps,
                                lhsT=wqk_t[:, cc, kk, :],
                                rhs=xT_sb[:, cc, nh * 512 : (nh + 1) * 512],
                                start=(cc == 0),
                                stop=(cc == CC - 1),
                            )
                        nc.vector.tensor_copy(
                            out=qk_t[:, kk, nh * 512 : (nh + 1) * 512], in_=ps
                        )
            return qk_t, wqk_t

        # first pair's qk immediately (only needs xT half 0 + its wqk cols)
        pair0 = emit_qk_pair(0, nhs=(0,))
        for cc in range(CC):
            nc.sync.dma_start(
                out=xT_sb[:, cc, 512:1024],
                in_=xT_d[cc * P : (cc + 1) * P, 512:1024],
            )
        # pol columns of v_aug: tiny broadcast DMAs from DRAM
        for nch in range(NC_):
            nc.sync.dma_start(
                out=v_aug[:, nch, :, HD : HD + 1],
                in_=polT_d[:, nch : nch + 1]
                .unsqueeze(1)
                .broadcast_to((P, H, 1))
                .bitcast(F32R),
            )

        # second half of pair 0 now that xT half 1 is queued
        qk_pair, _ = emit_qk_pair(0, nhs=(1,), tiles=pair0)

        # ---- v natural layout, pol-scaled, into v_aug --------------------
        with tc.tile_pool(name="phv", bufs=1) as phv:
            wv_sb = phv.tile([P, CC, H * HD], F32R)
            for cc in range(CC):
                nc.sync.dma_start(
                    out=wv_sb[:, cc], in_=wvT_d[cc * P : (cc + 1) * P, :]
                )
            with nc.named_scope("v_mm"):
                for nch in range(NC_):
                    for ev0, ev_sz, h0 in ((0, 512, 0), (512, 256, 8)):
                        nheads = ev_sz // HD
                        ps = ps_st.tile([P, 2, 512], F32, tag="st", name="ps_v")
                        psv = ps[:, 0, :ev_sz]
                        for cc in range(CC):
                            nc.tensor.matmul(
                                psv,
                                lhsT=xT_sb[:, cc, nch * P : (nch + 1) * P],
                                rhs=wv_sb[:, cc, ev0 : ev0 + ev_sz],
                                start=(cc == 0),
                                stop=(cc == CC - 1),
                            )
                        nc.vector.tensor_mul(
                            out=v_aug[:, nch, h0 : h0 + nheads, 0:HD],
                            in0=psv.rearrange("p (h d) -> p h d", d=HD),
                            in1=polT_sb[:, nch : nch + 1]
                            .unsqueeze(1)
                            .broadcast_to((P, nheads, HD)),
                        )

        # ------------------- attention, head-major ------------------------
        with tc.tile_pool(name="attn", bufs=1) as attn:
            outT = attn.tile([P, CC, N], F32R)
            pw_dma_done = False
            av_sbs = {}
            dens = {}
            for h in range(H):
                hp = 64 * (h % 2)
                qj = h // 2
                if h % 2 == 0 and h // 2 + 1 < CC:
                    nxt, _ = emit_qk_pair(h // 2 + 1)  # prefetch next pair
                elif h == H - 2:
                    nxt = None
                if h % 3 == 0:
                    dens[h // 3] = attn.tile(
                        [6, 512], F32, tag="den", bufs=2, name="den6"
                    )
                for nh in range(NH):
                    nsl = slice(nh * 512, (nh + 1) * 512)
                    av = ps_av.tile([HD + 1, 512], F32, tag="av", name="av")
                    E_ts = []
                    # chunk-pair pipeline: st(t) || exp(t-1..2) || av(t-3)
                    for t in range(NC_ // 2 + 3):
                        if t < NC_ // 2:
                            st = ps_st.tile([P, 2, 512], F32, tag="st", name="st")
                            E_t = attn.tile(
                                [P, 2, 512], F32R, tag="E", bufs=5, name="E_t"
                            )
                            with nc.named_scope("st_mm"):
                                for k in range(2):
                                    mc = 2 * t + k
                                    nc.tensor.matmul(
                                        st[:, k, :],
                                        lhsT=qk_pair[
                                            hp : hp + HD, 1, mc * P : (mc + 1) * P
                                        ],
                                        rhs=qk_pair[hp : hp + HD, 0, nsl],
                                        start=True,
                                        stop=True,
                                    )
                            nc.scalar.activation(
                                out=E_t,
                                in_=st,
                                func=mybir.ActivationFunctionType.Exp,
                                scale=SCALE,
                            )
                            if t in (2 * nh, 2 * nh + 1):
                                # chunks 2t, 2t+1 hold the diagonal
                                diag_ap = bass.AP(
                                    tensor=E_t.tensor,
                                    offset=E_t.offset + (2 * t * P - 512 * nh),
                                    ap=[E_t.ap[0], [512 + P, 2], [1, P]],
                                )
                                nc.vector.tensor_mul(
                                    out=diag_ap,
                                    in0=diag_ap,
                                    in1=dmask_sb[:, 2 * t : 2 * t + 2, :],
                                )
                            E_ts.append(E_t)
                        if t >= 3:
                            ta = t - 3
                            E_a = E_ts[ta]
                            with nc.named_scope("av_mm"):
                                for k in range(2):
                                    mc = 2 * ta + k
                                    nc.tensor.matmul(
                                        av,
                                        lhsT=v_aug[:, mc, h, :],
                                        rhs=E_a[:, k, :],
                                        start=(mc == 0),
                                        stop=(mc == NC_ - 1),
                                    )
                    av_sb = attn.tile(
                        [HD + 1, 512], F32, tag="avsb", bufs=7, name="av_sb"
                    )
                    nc.vector.tensor_copy(out=av_sb, in_=av)
                    av_sbs[(h, nh)] = av_sb
                    # gather denominators: 3 heads x 2 halves per batch
                    nc.sync.dma_start(
                        out=dens[h // 3][(h % 3) * 2 + nh : (h % 3) * 2 + nh + 1, :],
                        in_=av_sb[HD : HD + 1, :],
                    )
                if h % 2 == 0 and h // 2 + 1 < CC:
                    pass
                if h % 2 == 1:
                    qk_pair = nxt
                if h == 0 and not pw_dma_done:
                    # projection weights: stream in once the input burst ends
                    for cc in range(CC):
                        nc.sync.dma_start(
                            out=pw_sb[:, cc], in_=pwT_d[cc * P : (cc + 1) * P, :]
                        )
                    pw_dma_done = True
                if h % 3 == 2:
                    # batched reciprocal + normalization for 3 heads x 2 nh
                    g = h // 3
                    den6 = dens[g]
                    nc.vector.reciprocal(out=den6, in_=den6)
                    r_dram = rdramp.tile([6, 512], F32, tag="rd", name="r_dram")
                    nc.sync.dma_start(out=r_dram, in_=den6)
                    with nc.named_scope("norm"):
                        for i in range(6):
                            hh = 3 * g + i // 2
                            nhh = i % 2
                            nsl2 = slice(nhh * 512, (nhh + 1) * 512)
                            qj2 = hh // 2
                            r_bc = attn.tile(
                                [HD, 512], F32, tag="rbc", bufs=2, name="r_bc"
                            )
                            nc.sync.dma_start(
                                out=r_bc,
                                in_=r_dram[i].partition_broadcast(HD),
                            )
                            if hh % 2 == 0:
                                nc.vector.tensor_mul(
                                    out=outT[0:HD, qj2, nsl2],
                                    in0=av_sbs[(hh, nhh)][0:HD, :],
                                    in1=r_bc,
                                )
                            else:
                                tmp = attn.tile(
                                    [HD, 512], F32R, tag="otmp", bufs=2, name="tmp"
                                )
                                nc.vector.tensor_mul(
                                    out=tmp, in0=av_sbs[(hh, nhh)][0:HD, :], in1=r_bc
                                )
                                nc.sync.dma_start(
                                    out=outT[HD:P, qj2, nsl2], in_=tmp
                                )

            # --------------------------- projection -----------------------
            with nc.named_scope("proj_mm"):
                for nch in range(NC_):
                    y_t = attn.tile([P, C], F32, tag="y", bufs=1, name="y_t")
                    for oi, (o0, o_sz) in enumerate(((0, 512), (512, 256))):
                        pool = ps_qk if (2 * nch + oi) % 2 == 0 else ps_av
                        tagn = "ps_qk" if pool is ps_qk else "av"
                        ps = pool.tile([P, 512], F32, tag=tagn, name="ps_yt")
                        psy = ps[:, :o_sz]
                        for ec in range(CC):
                            nc.tensor.matmul(
                                psy,
                                lhsT=outT[:, ec, nch * P : (nch + 1) * P],
                                rhs=pw_sb[:, ec, o0 : o0 + o_sz],
                                start=(ec == 0),
                                stop=(ec == CC - 1),
                            )
                        nc.vector.tensor_add(
                            out=y_t[:, o0 : o0 + o_sz],
                            in0=psy,
                            in1=b_sb[:, o0 : o0 + o_sz],
                        )
                    nc.sync.dma_start(out=y_d[nch * P : (nch + 1) * P, :], in_=y_t)

    nc.compile()
    return nc


_NC_CACHE = None


def _get_nc():
    global _NC_CACHE
    if _NC_CACHE is None:
        _NC_CACHE = _build_nc()
    return _NC_CACHE


def kernel(x, policy, qkv_w, proj_w, proj_b):
    global LAST_RESULTS
    x = np.asarray(x, dtype=np.float32)
    policy = np.asarray(policy, dtype=np.float32)
    qkv_w = np.asarray(qkv_w, dtype=np.float32)
    proj_w = np.asarray(proj_w, dtype=np.float32)
    proj_b = np.asarray(proj_b, dtype=np.float32)

    wqkT = np.ascontiguousarray(qkv_w[: 2 * H * HD].T)  # [768, 1536]
    wvT = np.ascontiguousarray(qkv_w[2 * H * HD :].T)  # [768, 768]
    pwT = np.ascontiguousarray(proj_w.T)  # [768, 768]

    in_maps = []
    for b in range(B):
        pol = policy[b, :, 0]
        polc = np.maximum(pol, 1e-30)
        # [p, chunk] layout: global n = chunk*128 + p
        polT = np.ascontiguousarray(pol.reshape(NC_, P).T)
        dmask = np.ones((P, NC_, P), dtype=np.float32)
        rng = np.arange(P)
        for kch in range(NC_):
            dmask[rng, kch, rng] = 1.0 / polc[kch * P + rng]
        in_maps.append(
            dict(
                xT=np.ascontiguousarray(x[b].T),
                wqkT=wqkT,
                wvT=wvT,
                pwT=pwT,
                bias=proj_b,
                polT=polT.astype(np.float32),
                dmask=dmask,
            )
        )

    nc = _get_nc()
    trace = os.environ.get("KERNEL_TRACE", "0") == "1"
    res = run_bass_kernel_spmd(
        nc,
        in_maps,
        core_ids=list(range(B)),
        trace=trace,
        trace_cores=list(range(B)) if trace else None,
        stitch_traces=False,
    )
    LAST_RESULTS = res
    return np.stack([res.results[b]["y"] for b in range(B)], axis=0)



# revision 8
# speedup vs baseline: 1.0061x; 1.0061x over previous
"""Trainium2 Bass kernel for policy-weighted multi-head attention.

Reference computation (per batch b, 8 batches):
    qkv = x @ qkv_w.T                     # [N, 3*H*HD]
    q, k, v per head                      # H=12 heads, HD=64
    s = (q * HD^-0.5) @ k.T               # [N, N]
    a[n,m] ~ exp(s[n,m]) * (pol[m] + (1-pol[m])*eye)  normalized over m
    out = a @ v ; y = out @ proj_w.T + b

Sharding: pure data parallel, one batch per NeuronCore (8 cores).

Kernel strategy (per core):
  - Host pre-transposes x, qkv_w, proj_w so no on-chip transposes are needed.
  - All matmuls run as float32r (full fp32 data, streams ~250ns per
    [128x128x512] on the PE; ~2e-5 mean rel err per 128-dot).
  - Attention runs in the S^T layout (partitions = key index m): the softmax
    sum over m folds into the PE via an appended ones column on the
    (policy-prescaled) V; the denominator appears as row 64 of the
    attention-output matmul.
  - The policy multiply is folded into V (rows pre-scaled by pol[m]); the
    diagonal term becomes masked multiplies with a precomputed
    [128, 8, 128] mask whose diagonal is 1/pol.
  - Head-major processing with rotating qk buffers: the qk^T matmuls for
    head pair j+1 are emitted (and their weights DMA'd) while pair j's
    attention runs, so only 3 q/k pair buffers are resident.
  - The per-head dataflow is software-pipelined for the in-order engine
    queues: S^T matmuls for chunk-pair t overlap the exp of pair t-1 and
    the a@v matmuls of pair t-2.
  - Denominator reciprocals are batched (3 heads x 2 halves at a time:
    gathered onto 6 partitions by SBUF->SBUF DMA, one DVE reciprocal,
    broadcast back via a DRAM bounce -- DMA cannot partition-broadcast
    from SBUF).
  - max-subtraction and the eps terms of the reference softmax are dropped:
    logits are ~N(0,1) so exp() cannot overflow, and the eps corrections
    are ~1e-9 relative -- far below fp32 noise.
"""

import os

os.environ.setdefault("JAX_PLATFORMS", "axon")

from contextlib import ExitStack

import ml_dtypes
import numpy as np

import concourse.bass as bass
import concourse.tile as tile
from concourse import bacc, mybir
from concourse.bass_utils import run_bass_kernel_spmd

B, N, C = 8, 1024, 768
H, HD = 12, 64
SCALE = HD ** (-0.5)
F32 = mybir.dt.float32
F32R = mybir.dt.float32r
BF16 = mybir.dt.bfloat16
P = 128
NC_ = N // P  # 8 seq chunks
CC = C // P  # 6 channel chunks
NH = N // 512  # 2 free-dim halves of the seq axis

LAST_RESULTS = None  # BassKernelResults of the most recent run (for test.py)


def _build_nc():
    nc = bacc.Bacc(None, target_bir_lowering=False)

    xT_d = nc.dram_tensor("xT", [C, N], F32R, kind="ExternalInput")
    wqkT_d = nc.dram_tensor("wqkT", [C, 2 * H * HD], F32R, kind="ExternalInput")
    wvT_d = nc.dram_tensor("wvT", [C, H * HD], F32R, kind="ExternalInput")
    pwT_d = nc.dram_tensor("pwT", [C, C], F32R, kind="ExternalInput")
    bias_d = nc.dram_tensor("bias", [C], F32, kind="ExternalInput")
    polT_d = nc.dram_tensor("polT", [P, NC_], F32, kind="ExternalInput")
    dmask_d = nc.dram_tensor("dmask", [P, NC_, P], F32R, kind="ExternalInput")
    y_d = nc.dram_tensor("y", [N, C], F32, kind="ExternalOutput")

    with ExitStack() as ctx:
        tc = ctx.enter_context(tile.TileContext(nc))

        persist = ctx.enter_context(tc.tile_pool(name="persist", bufs=1))
        xT_sb = persist.tile([P, CC, N], F32R)
        # v in natural layout, pol-scaled, with a pol column at d=64
        v_aug = persist.tile([P, NC_, H, HD + 1], F32R)
        pw_sb = persist.tile([P, CC, C], F32R)
        b_sb = persist.tile([P, C], F32)
        polT_sb = persist.tile([P, NC_], F32)
        dmask_sb = persist.tile([P, NC_, P], F32R)

        # rotating q/k buffers: one pair j holds q chunk j and k chunk j+6
        qkp = ctx.enter_context(tc.tile_pool(name="qkp", bufs=1))
        ps_qk = ctx.enter_context(tc.tile_pool(name="ps_qk", bufs=1, space="PSUM"))
        ps_st = ctx.enter_context(tc.tile_pool(name="ps_st", bufs=3, space="PSUM"))
        ps_av = ctx.enter_context(tc.tile_pool(name="ps_av", bufs=1, space="PSUM"))
        rdramp = ctx.enter_context(tc.tile_pool(name="rdram", bufs=2, space="DRAM"))

        # ---- priority startup: pair-0 qk deps land first, cc-interleaved ----
        # The sync HWDGE ring drains in program order, so emission order is
        # effective DMA priority: the first matmul's tiles arrive in ~2us
        # instead of behind the full 3MB startup burst.
        wqk0_t = qkp.tile([P, CC, 2, P], F32R, tag="wqk", bufs=3, name="wqk_t")
        for cc in range(CC):
            for kk, j in ((0, 0), (1, CC)):
                nc.sync.dma_start(
                    out=wqk0_t[:, cc, kk, :],
                    in_=wqkT_d[cc * P : (cc + 1) * P, j * P : (j + 1) * P],
                )
            nc.sync.dma_start(
                out=xT_sb[:, cc, 0:512], in_=xT_d[cc * P : (cc + 1) * P, 0:512]
            )
        # HAM warm-up: the PE clock-gate defaults to 1.2GHz and only reaches
        # 2.4GHz after ~3.4us of sustained matmul activity.  Burn dummy
        # matmuls on the first-arriving weight tile while the rest of the
        # startup burst streams in, so real matmuls start at full clock.
        with nc.named_scope("warmup"):
            ps_w = ps_qk.tile([P, 512], F32, tag="ps_qk", name="ps_warm")
            for _ in range(56):
                nc.tensor.matmul(
                    ps_w[:, 0:128],
                    lhsT=wqk0_t[:, 0, 0, :],
                    rhs=wqk0_t[:, 0, 0, :],
                    start=True,
                    stop=True,
                )

        def emit_qk_pair(jq, nhs=(0, 1), tiles=None):
            """Load wqk columns for pair jq and run its qk^T matmul groups
            for the given seq halves.

            Returns (qk_t, wqk_t); qk_t is the [P, 2, N] tile: [:,0,:] =
            q-chunk jq, [:,1,:] = k-chunk jq+6 (e on partitions)."""
            if tiles is None:
                wqk_t = qkp.tile(
                    [P, CC, 2, P], F32R, tag="wqk", bufs=3, name="wqk_t"
                )
                for kk, j in ((0, jq), (1, jq + CC)):
                    for cc in range(CC):
                        nc.sync.dma_start(
                            out=wqk_t[:, cc, kk, :],
                            in_=wqkT_d[cc * P : (cc + 1) * P, j * P : (j + 1) * P],
                        )
                qk_t = qkp.tile([P, 2, N], F32R, tag="qkT", bufs=3, name="qk_t")
            else:
                qk_t, wqk_t = tiles
            with nc.named_scope("qk_mm"):
                for kk in range(2):
                    for nh in nhs:
                        ps = ps_qk.tile([P, 512], F32, tag="ps_qk", name="ps_qk")
                        for cc in range(CC):
                            nc.tensor.matmul(
                                ps,
                                lhsT=wqk_t[:, cc, kk, :],
                                rhs=xT_sb[:, cc, nh * 512 : (nh + 1) * 512],
                                start=(cc == 0),
                                stop=(cc == CC - 1),
                            )
                        nc.vector.tensor_copy(
                            out=qk_t[:, kk, nh * 512 : (nh + 1) * 512], in_=ps
                        )
            return qk_t, wqk_t

        # first pair's qk immediately (only needs xT half 0 + its wqk cols)
        qk0_t = qkp.tile([P, 2, N], F32R, tag="qkT", bufs=3, name="qk_t")
        pair0 = emit_qk_pair(0, nhs=(0,), tiles=(qk0_t, wqk0_t))
        for cc in range(CC):
            nc.sync.dma_start(
                out=xT_sb[:, cc, 512:1024],
                in_=xT_d[cc * P : (cc + 1) * P, 512:1024],
            )
        nc.sync.dma_start(out=polT_sb, in_=polT_d[:])
        # pol columns of v_aug: tiny broadcast DMAs, on the SWDGE ring so
        # they stay off the (priority-ordered) sync ring
        for nch in range(NC_):
            nc.gpsimd.dma_start(
                out=v_aug[:, nch, :, HD : HD + 1],
                in_=polT_d[:, nch : nch + 1]
                .unsqueeze(1)
                .broadcast_to((P, H, 1))
                .bitcast(F32R),
            )

        # ---- v natural layout, pol-scaled, into v_aug --------------------
        with tc.tile_pool(name="phv", bufs=1) as phv:
            wv_sb = phv.tile([P, CC, H * HD], F32R)
            for cc in range(CC):
                nc.sync.dma_start(
                    out=wv_sb[:, cc], in_=wvT_d[cc * P : (cc + 1) * P, :]
                )
            nc.sync.dma_start(out=dmask_sb, in_=dmask_d[:])
            nc.sync.dma_start(out=b_sb, in_=bias_d[:].partition_broadcast(P))

            # second half of pair 0 now that xT half 1 is queued
            qk_pair, _ = emit_qk_pair(0, nhs=(1,), tiles=pair0)

            with nc.named_scope("v_mm"):
                for nch in range(NC_):
                    for ev0, ev_sz, h0 in ((0, 512, 0), (512, 256, 8)):
                        nheads = ev_sz // HD
                        ps = ps_st.tile([P, 2, 512], F32, tag="st", name="ps_v")
                        psv = ps[:, 0, :ev_sz]
                        for cc in range(CC):
                            nc.tensor.matmul(
                                psv,
                                lhsT=xT_sb[:, cc, nch * P : (nch + 1) * P],
                                rhs=wv_sb[:, cc, ev0 : ev0 + ev_sz],
                                start=(cc == 0),
                                stop=(cc == CC - 1),
                            )
                        nc.vector.tensor_mul(
                            out=v_aug[:, nch, h0 : h0 + nheads, 0:HD],
                            in0=psv.rearrange("p (h d) -> p h d", d=HD),
                            in1=polT_sb[:, nch : nch + 1]
                            .unsqueeze(1)
                            .broadcast_to((P, nheads, HD)),
                        )

        # ------------------- attention, head-major ------------------------
        with tc.tile_pool(name="attn", bufs=1) as attn:
            outT = attn.tile([P, CC, N], F32R)
            pw_dma_done = False
            av_sbs = {}
            dens = {}
            for h in range(H):
                hp = 64 * (h % 2)
                qj = h // 2
                if h % 2 == 0 and h // 2 + 1 < CC:
                    nxt, _ = emit_qk_pair(h // 2 + 1)  # prefetch next pair
                elif h == H - 2:
                    nxt = None
                if h % 3 == 0:
                    dens[h // 3] = attn.tile(
                        [6, 512], F32, tag="den", bufs=2, name="den6"
                    )
                for nh in range(NH):
                    nsl = slice(nh * 512, (nh + 1) * 512)
                    av = ps_av.tile([HD + 1, 512], F32, tag="av", name="av")
                    E_ts = []
                    # chunk-pair pipeline: st(t) || exp(t-1..2) || av(t-3)
                    for t in range(NC_ // 2 + 3):
                        if t < NC_ // 2:
                            st = ps_st.tile([P, 2, 512], F32, tag="st", name="st")
                            E_t = attn.tile(
                                [P, 2, 512], F32R, tag="E", bufs=5, name="E_t"
                            )
                            with nc.named_scope("st_mm"):
                                for k in range(2):
                                    mc = 2 * t + k
                                    nc.tensor.matmul(
                                        st[:, k, :],
                                        lhsT=qk_pair[
                                            hp : hp + HD, 1, mc * P : (mc + 1) * P
                                        ],
                                        rhs=qk_pair[hp : hp + HD, 0, nsl],
                                        start=True,
                                        stop=True,
                                    )
                            nc.scalar.activation(
                                out=E_t,
                                in_=st,
                                func=mybir.ActivationFunctionType.Exp,
                                scale=SCALE,
                            )
                            if t in (2 * nh, 2 * nh + 1):
                                # chunks 2t, 2t+1 hold the diagonal
                                diag_ap = bass.AP(
                                    tensor=E_t.tensor,
                                    offset=E_t.offset + (2 * t * P - 512 * nh),
                                    ap=[E_t.ap[0], [512 + P, 2], [1, P]],
                                )
                                nc.vector.tensor_mul(
                                    out=diag_ap,
                                    in0=diag_ap,
                                    in1=dmask_sb[:, 2 * t : 2 * t + 2, :],
                                )
                            E_ts.append(E_t)
                        if t >= 3:
                            ta = t - 3
                            E_a = E_ts[ta]
                            with nc.named_scope("av_mm"):
                                for k in range(2):
                                    mc = 2 * ta + k
                                    nc.tensor.matmul(
                                        av,
                                        lhsT=v_aug[:, mc, h, :],
                                        rhs=E_a[:, k, :],
                                        start=(mc == 0),
                                        stop=(mc == NC_ - 1),
                                    )
                    av_sb = attn.tile(
                        [HD + 1, 512], F32, tag="avsb", bufs=7, name="av_sb"
                    )
                    nc.vector.tensor_copy(out=av_sb, in_=av)
                    av_sbs[(h, nh)] = av_sb
                    # gather denominators: 3 heads x 2 halves per batch
                    nc.gpsimd.dma_start(
                        out=dens[h // 3][(h % 3) * 2 + nh : (h % 3) * 2 + nh + 1, :],
                        in_=av_sb[HD : HD + 1, :],
                    )
                if h % 2 == 0 and h // 2 + 1 < CC:
                    pass
                if h % 2 == 1:
                    qk_pair = nxt
                if h == 0 and not pw_dma_done:
                    # projection weights: on the Act HWDGE ring, dispatched
                    # after h0's exps -- off the startup burst entirely
                    for cc in range(CC):
                        nc.scalar.dma_start(
                            out=pw_sb[:, cc], in_=pwT_d[cc * P : (cc + 1) * P, :]
                        )
                    pw_dma_done = True
                if h % 3 == 2:
                    # batched reciprocal + normalization for 3 heads x 2 nh
                    g = h // 3
                    den6 = dens[g]
                    nc.vector.reciprocal_approx_fast(out=den6, in_=den6)
                    r_dram = rdramp.tile([6, 512], F32, tag="rd", name="r_dram")
                    nc.gpsimd.dma_start(out=r_dram, in_=den6)
                    with nc.named_scope("norm"):
                        for i in range(6):
                            hh = 3 * g + i // 2
                            nhh = i % 2
                            nsl2 = slice(nhh * 512, (nhh + 1) * 512)
                            qj2 = hh // 2
                            r_bc = attn.tile(
                                [HD, 512], F32, tag="rbc", bufs=2, name="r_bc"
                            )
                            nc.gpsimd.dma_start(
                                out=r_bc,
                                in_=r_dram[i].partition_broadcast(HD),
                            )
                            if hh % 2 == 0:
                                nc.vector.tensor_mul(
                                    out=outT[0:HD, qj2, nsl2],
                                    in0=av_sbs[(hh, nhh)][0:HD, :],
                                    in1=r_bc,
                                )
                            else:
                                tmp = attn.tile(
                                    [HD, 512], F32R, tag="otmp", bufs=2, name="tmp"
                                )
                                nc.vector.tensor_mul(
                                    out=tmp, in0=av_sbs[(hh, nhh)][0:HD, :], in1=r_bc
                                )
                                nc.gpsimd.dma_start(
                                    out=outT[HD:P, qj2, nsl2], in_=tmp
                                )

            # --------------------------- projection -----------------------
            with nc.named_scope("proj_mm"):
                for nch in range(NC_):
                    y_t = attn.tile([P, C], F32, tag="y", bufs=1, name="y_t")
                    for oi, (o0, o_sz) in enumerate(((0, 512), (512, 256))):
                        pool = ps_qk if (2 * nch + oi) % 2 == 0 else ps_av
                        tagn = "ps_qk" if pool is ps_qk else "av"
                        ps = pool.tile([P, 512], F32, tag=tagn, name="ps_yt")
                        psy = ps[:, :o_sz]
                        for ec in range(CC):
                            nc.tensor.matmul(
                                psy,
                                lhsT=outT[:, ec, nch * P : (nch + 1) * P],
                                rhs=pw_sb[:, ec, o0 : o0 + o_sz],
                                start=(ec == 0),
                                stop=(ec == CC - 1),
                            )
                        nc.vector.tensor_add(
                            out=y_t[:, o0 : o0 + o_sz],
                            in0=psy,
                            in1=b_sb[:, o0 : o0 + o_sz],
                        )
                    nc.sync.dma_start(out=y_d[nch * P : (nch + 1) * P, :], in_=y_t)

    nc.compile()
    return nc


_NC_CACHE = None


def _get_nc():
    global _NC_CACHE
    if _NC_CACHE is None:
        _NC_CACHE = _build_nc()
    return _NC_CACHE


def kernel(x, policy, qkv_w, proj_w, proj_b):
    global LAST_RESULTS
    x = np.asarray(x, dtype=np.float32)
    policy = np.asarray(policy, dtype=np.float32)
    qkv_w = np.asarray(qkv_w, dtype=np.float32)
    proj_w = np.asarray(proj_w, dtype=np.float32)
    proj_b = np.asarray(proj_b, dtype=np.float32)

    wqkT = np.ascontiguousarray(qkv_w[: 2 * H * HD].T)  # [768, 1536]
    wvT = np.ascontiguousarray(qkv_w[2 * H * HD :].T)  # [768, 768]
    pwT = np.ascontiguousarray(proj_w.T)  # [768, 768]

    in_maps = []
    for b in range(B):
        pol = policy[b, :, 0]
        polc = np.maximum(pol, 1e-30)
        # [p, chunk] layout: global n = chunk*128 + p
        polT = np.ascontiguousarray(pol.reshape(NC_, P).T)
        dmask = np.ones((P, NC_, P), dtype=np.float32)
        rng = np.arange(P)
        for kch in range(NC_):
            dmask[rng, kch, rng] = 1.0 / polc[kch * P + rng]
        in_maps.append(
            dict(
                xT=np.ascontiguousarray(x[b].T),
                wqkT=wqkT,
                wvT=wvT,
                pwT=pwT,
                bias=proj_b,
                polT=polT.astype(np.float32),
                dmask=dmask,
            )
        )

    nc = _get_nc()
    trace = os.environ.get("KERNEL_TRACE", "0") == "1"
    res = run_bass_kernel_spmd(
        nc,
        in_maps,
        core_ids=list(range(B)),
        trace=trace,
        trace_cores=list(range(B)) if trace else None,
        stitch_traces=False,
    )
    LAST_RESULTS = res
    return np.stack([res.results[b]["y"] for b in range(B)], axis=0)



# revision 23
# speedup vs baseline: 1.1947x; 1.1875x over previous
"""Trainium2 Bass kernel for policy-weighted multi-head attention.

Reference computation (per batch b, 8 batches):
    qkv = x @ qkv_w.T                     # [N, 3*H*HD]
    q, k, v per head                      # H=12 heads, HD=64
    s = (q * HD^-0.5) @ k.T               # [N, N]
    a[n,m] ~ exp(s[n,m]) * (pol[m] + (1-pol[m])*eye)  normalized over m
    out = a @ v ; y = out @ proj_w.T + b

Sharding: pure data parallel, one batch per NeuronCore (8 cores).

Kernel strategy (per core):
  - Host pre-transposes x, qkv_w, proj_w so no on-chip transposes are needed.
  - All matmuls run as float32r (full fp32 data, ~250ns per [128x128x512]).
  - Attention runs in the S^T layout (partitions = key index m): the softmax
    sum over m folds into the PE via an appended ones column on the
    (policy-prescaled) V; the denominator appears as row 64 of the
    attention-output matmul.
  - The policy multiply is folded into V (rows pre-scaled by pol[m]); the
    diagonal term becomes masked multiplies with a precomputed
    [128, 8, 128] mask whose diagonal is 1/pol.
  - nh-major scheduling: the two 512-wide halves of the query axis are
    processed as outer passes over all 12 heads; all 6 q/k pair buffers stay
    resident (prefetched during pass 0), and the projection matmuls for
    pass-0 rows are interleaved into pass 1 so the PE never drains between
    attention and projection.
  - Normalization is DMA-free: per (head, half), 1/denominator via a single
    fast-approx DVE reciprocal on the av row, partition-broadcast by a K=1
    ones-matmul into PSUM, then one DVE multiply into the output tile.  Only
    the odd-head partition shift (rows 64..127) needs an SBUF->SBUF DMA, on
    the otherwise-idle SWDGE ring.  Norm emission is deferred into the next
    head's st loop so the PE never waits on the DVE reciprocal chain.
  - Bulk inputs are single-dispatch DMAs (each dma_start costs ~0.65us of
    sequencer dispatch); the first qk pair + x streams go on the Act HWDGE
    ring, which is idle during the sync ring's kernel preamble, and dummy
    warm-up matmuls run during the load so the PE clock-gate (HAM) reaches
    2.4GHz before real work starts.
  - max-subtraction and the eps terms of the reference softmax are dropped:
    logits are ~N(0,1) so exp() cannot overflow, and the eps corrections
    are ~1e-9 relative -- far below fp32 noise.
"""

import os

os.environ.setdefault("JAX_PLATFORMS", "axon")

from contextlib import ExitStack

import ml_dtypes
import numpy as np

import concourse.bass as bass
import concourse.tile as tile
from concourse import bacc, mybir
from concourse.bass_utils import run_bass_kernel_spmd

B, N, C = 8, 1024, 768
H, HD = 12, 64
SCALE = HD ** (-0.5)
F32 = mybir.dt.float32
F32R = mybir.dt.float32r
BF16 = mybir.dt.bfloat16
P = 128
NC_ = N // P  # 8 seq chunks
CC = C // P  # 6 channel chunks
NH = N // 512  # 2 free-dim halves of the seq axis

LAST_RESULTS = None  # BassKernelResults of the most recent run (for test.py)


def _build_nc():
    nc = bacc.Bacc(None, target_bir_lowering=False)

    xT_d = nc.dram_tensor("xT", [C, N], F32R, kind="ExternalInput")
    wqkT_d = nc.dram_tensor("wqkT", [C, 2 * H * HD], F32R, kind="ExternalInput")
    wvT_d = nc.dram_tensor("wvT", [C, H * HD], F32R, kind="ExternalInput")
    pwT_d = nc.dram_tensor("pwT", [C, C], F32R, kind="ExternalInput")
    bias_d = nc.dram_tensor("bias", [C], F32, kind="ExternalInput")
    polT_d = nc.dram_tensor("polT", [P, NC_], F32, kind="ExternalInput")
    dmask_d = nc.dram_tensor("dmask", [P, NC_, P], F32R, kind="ExternalInput")
    y_d = nc.dram_tensor("y", [N, C], F32, kind="ExternalOutput")

    def dram_cc(t_d, c0, w):
        # [768, w] DRAM slice viewed as [128, CC, w] for one-dispatch loads
        return t_d[:, c0 : c0 + w].rearrange("(cc p) w -> p cc w", p=P)

    with ExitStack() as ctx:
        tc = ctx.enter_context(tile.TileContext(nc))

        persist = ctx.enter_context(tc.tile_pool(name="persist", bufs=1))
        xT_sb = persist.tile([P, CC, N], F32R)
        # v in natural layout, pol-scaled, with a pol column at d=64
        v_aug = persist.tile([P, NC_, H, HD + 1], F32R)
        pw_sb = persist.tile([P, CC, C], F32R)
        b_sb = persist.tile([P, C], F32)
        polT_sb = persist.tile([P, NC_], F32)
        dmask_sb = persist.tile([P, NC_, P], F32R)
        e_f32 = persist.tile([P, HD], F32)
        e_sb = persist.tile([P, HD], F32R)  # unit row 64: selects 1/den
        z_f32 = persist.tile([P, 512], F32)
        rcp_rs = [persist.tile([P, 512], F32R, name=f"rcp_r{i}") for i in (0, 1)]
        outT = persist.tile([P, CC, N], F32R)

        qkp = ctx.enter_context(tc.tile_pool(name="qkp", bufs=1))
        ps_st = ctx.enter_context(tc.tile_pool(name="ps_st", bufs=3, space="PSUM"))
        ps_av = ctx.enter_context(tc.tile_pool(name="ps_av", bufs=1, space="PSUM"))
        ps_mx = ctx.enter_context(tc.tile_pool(name="ps_mx", bufs=1, space="PSUM"))

        nc.vector.memset(e_f32, 0.0)
        nc.vector.memset(e_f32[HD : HD + 1, :], 1.0)
        nc.vector.tensor_copy(out=e_sb, in_=e_f32)
        # rows 65..127 of the broadcast rhs must be real zeros: the K=128
        # matmul streams all 128 partitions on hardware
        nc.vector.memset(z_f32, 0.0)
        for t in rcp_rs:
            nc.vector.tensor_copy(out=t, in_=z_f32)

        # --- priority startup on the Act ring: its sequencer is idle while
        # the sync ring runs the kernel preamble (~7us), and each dma_start
        # costs ~0.65us of sequencer dispatch, so these are one-per-tensor.
        wqk_ts = {0: qkp.tile([P, CC, 2, P], F32R, tag="wqk", bufs=3, name="wqk_t")}
        qk_ts = {}
        for cc in range(CC):
            for kk, j in ((0, 0), (1, CC)):
                nc.sync.dma_start(
                    out=wqk_ts[0][:, cc, kk, :],
                    in_=wqkT_d[cc * P : (cc + 1) * P, j * P : (j + 1) * P],
                )
            nc.sync.dma_start(
                out=xT_sb[:, cc, 0:512], in_=xT_d[cc * P : (cc + 1) * P, 0:512]
            )
        # bulk on the sync ring, in need order
        nc.sync.dma_start(out=polT_sb, in_=polT_d[:])
        for cc in range(CC):
            nc.sync.dma_start(
                out=xT_sb[:, cc, 512:1024],
                in_=xT_d[cc * P : (cc + 1) * P, 512:1024],
            )
        nc.sync.dma_start(out=dmask_sb, in_=dmask_d[:])
        nc.sync.dma_start(out=b_sb, in_=bias_d[:].partition_broadcast(P))

        # HAM warm-up: the PE clock-gate defaults to 1.2GHz and only reaches
        # 2.4GHz after ~3.4us of sustained matmul activity.  Burn dummy
        # matmuls on the first-arriving weight tile while the startup burst
        # streams in, so real matmuls run at full clock.
        with nc.named_scope("warmup"):
            ps_w = ps_mx.tile([P, 512], F32, tag="mx", name="ps_warm")
            for _ in range(56):
                nc.tensor.matmul(
                    ps_w[:, 0:128],
                    lhsT=wqk_ts[0][:, 0, 0, :],
                    rhs=wqk_ts[0][:, 0, 0, :],
                    start=True,
                    stop=True,
                )

        def emit_qk_mms(jq, nhs=(0, 1)):
            """qk^T matmuls for pair jq: qk_t[:,0,:] = q chunk jq,
            qk_t[:,1,:] = k chunk jq+6 (embedding dim on partitions)."""
            wqk_t, qk_t = wqk_ts[jq], qk_ts[jq]
            with nc.named_scope("qk_mm"):
                for kk in range(2):
                    for nh in nhs:
                        ps = ps_mx.tile([P, 512], F32, tag="mx", name="ps_qk")
                        for cc in range(CC):
                            nc.tensor.matmul(
                                ps,
                                lhsT=wqk_t[:, cc, kk, :],
                                rhs=xT_sb[:, cc, nh * 512 : (nh + 1) * 512],
                                start=(cc == 0),
                                stop=(cc == CC - 1),
                            )
                        nc.scalar.activation(
                            out=qk_t[:, kk, nh * 512 : (nh + 1) * 512],
                            in_=ps,
                            func=mybir.ActivationFunctionType.Copy,
                        )

        qk_ts[0] = qkp.tile([P, 2, N], BF16, tag="qkT", bufs=CC, name="qk_t")
        emit_qk_mms(0, nhs=(0,))
        # pol columns of v_aug: DVE free-dim broadcast copies
        for nch in range(NC_):
            nc.vector.tensor_copy(
                out=v_aug[:, nch, :, HD : HD + 1],
                in_=polT_sb[:, nch : nch + 1]
                .unsqueeze(1)
                .broadcast_to((P, H, 1)),
            )
        emit_qk_mms(0, nhs=(1,))

        # ---- v natural layout, pol-scaled, into v_aug --------------------
        with tc.tile_pool(name="phv", bufs=1) as phv:
            wv_sb = phv.tile([P, CC, H * HD], F32R)
            for cc in range(CC):
                nc.sync.dma_start(
                    out=wv_sb[:, cc], in_=wvT_d[cc * P : (cc + 1) * P, :]
                )
            with nc.named_scope("v_mm"):
                for nch in range(NC_):
                    for ev0, ev_sz, h0 in ((0, 512, 0), (512, 256, 8)):
                        nheads = ev_sz // HD
                        ps = ps_st.tile([P, 2, 512], F32, tag="st", name="ps_v")
                        psv = ps[:, 0, :ev_sz]
                        for cc in range(CC):
                            nc.tensor.matmul(
                                psv,
                                lhsT=xT_sb[:, cc, nch * P : (nch + 1) * P],
                                rhs=wv_sb[:, cc, ev0 : ev0 + ev_sz],
                                start=(cc == 0),
                                stop=(cc == CC - 1),
                            )
                        nc.vector.tensor_mul(
                            out=v_aug[:, nch, h0 : h0 + nheads, 0:HD],
                            in0=psv.rearrange("p (h d) -> p h d", d=HD),
                            in1=polT_sb[:, nch : nch + 1]
                            .unsqueeze(1)
                            .broadcast_to((P, nheads, HD)),
                        )

        # ------------------- attention, nh-major --------------------------
        with tc.tile_pool(name="attn", bufs=1) as attn:
            pending_norm = [None]  # deferred norm closure for the prev head

            def proj_chunk(nch):
                with nc.named_scope("proj_mm"):
                    y_t = attn.tile([P, C], F32, tag="y", bufs=2, name="y_t")
                    for oi, (o0, o_sz) in enumerate(((0, 512), (512, 256))):
                        pool, tagn = (ps_mx, "mx") if oi == 0 else (ps_av, "av")
                        ps = pool.tile([P, 512], F32, tag=tagn, name="ps_yt")
                        psy = ps[:, :o_sz]
                        for ec in range(CC):
                            nc.tensor.matmul(
                                psy,
                                lhsT=outT[:, ec, nch * P : (nch + 1) * P],
                                rhs=pw_sb[:, ec, o0 : o0 + o_sz],
                                start=(ec == 0),
                                stop=(ec == CC - 1),
                            )
                        nc.vector.tensor_add(
                            out=y_t[:, o0 : o0 + o_sz],
                            in0=psy,
                            in1=b_sb[:, o0 : o0 + o_sz],
                        )
                    nc.sync.dma_start(out=y_d[nch * P : (nch + 1) * P, :], in_=y_t)

            for nh in range(NH):
                nsl = slice(nh * 512, (nh + 1) * 512)
                for h in range(H):
                    j = h // 2
                    hp = 64 * (h % 2)
                    if nh == 0 and h % 2 == 0 and j + 1 < CC:
                        # prefetch next q/k pair (DMA + matmuls) during pass 0
                        jn = j + 1
                        wqk_ts[jn] = qkp.tile(
                            [P, CC, 2, P], F32R, tag="wqk", bufs=3, name="wqk_t"
                        )
                        for kk, jj in ((0, jn), (1, jn + CC)):
                            for cc in range(CC):
                                nc.sync.dma_start(
                                    out=wqk_ts[jn][:, cc, kk, :],
                                    in_=wqkT_d[
                                        cc * P : (cc + 1) * P,
                                        jj * P : (jj + 1) * P,
                                    ],
                                )
                        qk_ts[jn] = qkp.tile(
                            [P, 2, N], BF16, tag="qkT", bufs=CC, name="qk_t"
                        )
                        emit_qk_mms(jn)
                    qk_pair = qk_ts[j]
                    av = ps_av.tile([HD + 1, 512], F32, tag="av", name="av")
                    E_ts = []
                    # chunk-pair pipeline: st(t) || exp(t-1..2) || av(t-3)
                    for t in range(NC_ // 2 + 3):
                        if t == 3 and pending_norm[0] is not None:
                            # prev head's norm, emitted once this head's st
                            # matmuls are queued so the PE never waits on it
                            pending_norm[0]()
                            pending_norm[0] = None
                        if t < NC_ // 2:
                            st = ps_st.tile([P, 2, 512], F32, tag="st", name="st")
                            E_t = attn.tile(
                                [P, 2, 512], F32R, tag="E", bufs=5, name="E_t"
                            )
                            with nc.named_scope("st_mm"):
                                for k in range(2):
                                    mc = 2 * t + k
                                    nc.tensor.matmul(
                                        st[:, k, :],
                                        lhsT=qk_pair[
                                            hp : hp + HD, 1, mc * P : (mc + 1) * P
                                        ],
                                        rhs=qk_pair[hp : hp + HD, 0, nsl],
                                        start=True,
                                        stop=True,
                                    )
                            nc.scalar.activation(
                                out=E_t,
                                in_=st,
                                func=mybir.ActivationFunctionType.Exp,
                                scale=SCALE,
                            )
                            if t in (2 * nh, 2 * nh + 1):
                                # chunks 2t, 2t+1 hold the diagonal
                                diag_ap = bass.AP(
                                    tensor=E_t.tensor,
                                    offset=E_t.offset + (2 * t * P - 512 * nh),
                                    ap=[E_t.ap[0], [512 + P, 2], [1, P]],
                                )
                                nc.vector.tensor_mul(
                                    out=diag_ap,
                                    in0=diag_ap,
                                    in1=dmask_sb[:, 2 * t : 2 * t + 2, :],
                                )
                            E_ts.append(E_t)
                        if t >= 3:
                            ta = t - 3
                            E_a = E_ts[ta]
                            with nc.named_scope("av_mm"):
                                for k in range(2):
                                    mc = 2 * ta + k
                                    nc.tensor.matmul(
                                        av,
                                        lhsT=v_aug[:, mc, h, :],
                                        rhs=E_a[:, k, :],
                                        start=(mc == 0),
                                        stop=(mc == NC_ - 1),
                                    )
                    av_sb = attn.tile(
                        [HD + 1, 512], F32, tag="avsb", bufs=3, name="av_sb"
                    )
                    nc.vector.tensor_copy(out=av_sb, in_=av)

                    def make_norm(av_sb=av_sb, h=h, nh=nh, nsl=nsl):
                        def norm():
                            with nc.named_scope("norm"):
                                # rcp rows 0..63 are finite filler (the av
                                # values) so the K=65 broadcast matmul below
                                # never multiplies 0 by uninitialized bits
                                rcp = attn.tile(
                                    [HD + 1, 512], F32, tag="rcp", bufs=2,
                                    name="rcp",
                                )
                                # full-tile op from partition 0: the custom
                                # DVE op returns zeros on HW when started at
                                # a partition offset.  Rows 0..63 are 1/av --
                                # finite filler, zeroed by the selector.
                                nc.vector.reciprocal_approx_fast(
                                    out=rcp, in_=av_sb
                                )
                                # round to f32r on the (lightly loaded) Act
                                # engine; the f32r matmul requires it
                                rcp_r = rcp_rs[h % 2]
                                nc.scalar.activation(
                                    out=rcp_r[0 : HD + 1, :],
                                    in_=rcp,
                                    func=mybir.ActivationFunctionType.Copy,
                                )
                                r_ps = ps_mx.tile(
                                    [P, 512], F32, tag="mx", name="r_ps"
                                )
                                nc.tensor.matmul(
                                    r_ps[0:HD, :],
                                    lhsT=e_sb,
                                    rhs=rcp_r,
                                    start=True,
                                    stop=True,
                                )
                                qj = h // 2
                                if h % 2 == 0:
                                    nc.vector.tensor_mul(
                                        out=outT[0:HD, qj, nsl],
                                        in0=av_sb[0:HD, :],
                                        in1=r_ps[0:HD, :],
                                    )
                                else:
                                    tmp = attn.tile(
                                        [HD, 512], F32R, tag="otmp", bufs=2,
                                        name="tmp",
                                    )
                                    nc.vector.tensor_mul(
                                        out=tmp,
                                        in0=av_sb[0:HD, :],
                                        in1=r_ps[0:HD, :],
                                    )
                                    nc.sync.dma_start(
                                        out=outT[HD:P, qj, nsl], in_=tmp
                                    )

                        return norm

                    pending_norm[0] = make_norm()
                    if nh == 0 and h == 0:
                        # projection weights: one dispatch on the Act ring,
                        # after h0's exps -- off the startup burst entirely
                        for cc in range(CC):
                            nc.sync.dma_start(
                                out=pw_sb[:, cc],
                                in_=pwT_d[cc * P : (cc + 1) * P, :],
                            )
                    if nh == 1 and h % 3 == 1:
                        # interleave pass-0 projection rows into pass 1
                        proj_chunk(h // 3)
                # (pending norm for h==11 is flushed inside the next pass /
                # tail below)
            pending_norm[0]()
            pending_norm[0] = None
            for nch in range(4, NC_):
                proj_chunk(nch)

    nc.compile()
    return nc


_NC_CACHE = None


def _get_nc():
    global _NC_CACHE
    if _NC_CACHE is None:
        _NC_CACHE = _build_nc()
    return _NC_CACHE


def kernel(x, policy, qkv_w, proj_w, proj_b):
    global LAST_RESULTS
    x = np.asarray(x, dtype=np.float32)
    policy = np.asarray(policy, dtype=np.float32)
    qkv_w = np.asarray(qkv_w, dtype=np.float32)
    proj_w = np.asarray(proj_w, dtype=np.float32)
    proj_b = np.asarray(proj_b, dtype=np.float32)

    wqkT = np.ascontiguousarray(qkv_w[: 2 * H * HD].T)  # [768, 1536]
    wvT = np.ascontiguousarray(qkv_w[2 * H * HD :].T)  # [768, 768]
    pwT = np.ascontiguousarray(proj_w.T)  # [768, 768]

    in_maps = []
    for b in range(B):
        pol = policy[b, :, 0]
        polc = np.maximum(pol, 1e-30)
        # [p, chunk] layout: global n = chunk*128 + p
        polT = np.ascontiguousarray(pol.reshape(NC_, P).T)
        dmask = np.ones((P, NC_, P), dtype=np.float32)
        rng = np.arange(P)
        for kch in range(NC_):
            dmask[rng, kch, rng] = 1.0 / polc[kch * P + rng]
        in_maps.append(
            dict(
                xT=np.ascontiguousarray(x[b].T),
                wqkT=wqkT,
                wvT=wvT,
                pwT=pwT,
                bias=proj_b,
                polT=polT.astype(np.float32),
                dmask=dmask,
            )
        )

    nc = _get_nc()
    trace = os.environ.get("KERNEL_TRACE", "0") == "1"
    res = run_bass_kernel_spmd(
        nc,
        in_maps,
        core_ids=list(range(B)),
        trace=trace,
        trace_cores=list(range(B)) if trace else None,
        stitch_traces=False,
    )
    LAST_RESULTS = res
    return np.stack([res.results[b]["y"] for b in range(B)], axis=0)
